# revision 1
# baseline (speedup 1.0000x reference)
"""GCN-LSTM regressor v3 — gates computed fully in PSUM.

Structural change vs v2: gates_x is never materialized.  Each step's gate
matmul accumulates over BOTH k-spaces:
    gates[m, n] = sum_k whh[k,m] h[k,n] + sum_j wihm[j,m] h2c[j, s, n]
where h2c = relu([p; q; 1; mixed]) per chunk (k=256, the ones-row carries
the bias).  Everything is scaled by 512 (fp8-friendly) and the activation's
free `scale` param divides it back out.  This removes the STT, the gx
PSUM->SBUF evacuation, and the gx SBUF tiles entirely — the ACT engine was
77% busy in v2's trace and most of that work is gone.

Gate pairs share PSUM banks: [F|I] in one (single sigmoid over FD=480),
[G|O] in another.  The h2-part matmuls don't depend on h, so with
double-buffered gate banks the scheduler hoists them into the previous
step's activation tail.
"""

import sys

sys.path.insert(0, "/opt/trn_rl_repo")

import numpy as np
import ml_dtypes

import concourse.bass as bass
import concourse.mybir as mybir
import concourse.tile as tile
from concourse import bacc
from concourse.bass_utils import run_bass_kernel_spmd
from concourse.masks import make_identity

BF16 = ml_dtypes.bfloat16
FP8 = ml_dtypes.float8_e4m3
F32 = mybir.dt.float32
BF = mybir.dt.bfloat16
F8 = mybir.dt.float8e4
WSCALE = 512.0
AF = mybir.ActivationFunctionType
ALU = mybir.AluOpType

B, T, L, G, HL, OUT = 4, 192, 120, 500, 500, 24
NCORES = 8
NLOC = 60
CH = 8
NCH = T // CH
P = 128
KT = 4
MT = 16
KGX = 2  # h2 k-tiles (packed 256 = p,q,ones + 253 mixed)
HLP, H4P = 512, 2048
KXP = KGX * P
F1, F2, F3 = 3072, 1024, 3072
NPOS = CH * NLOC

WIHM_FP8 = True  # fp8 stationaries keep LDWEIGHTS off the critical pace


def _build_program(wihm_fp8=False):
    nc = bacc.Bacc(
        "TRN2",
        target_bir_lowering=False,
        debug=False,
        enable_asserts=True,
        num_devices=NCORES,
    )
    WIHM_DT = F8 if wihm_fp8 else BF

    xb = nc.declare_dram_parameter("xb", [2, 96, L], F32, isOutput=False)
    a_hat = nc.declare_dram_parameter("a_hat", [L, L], F32, isOutput=False)
    uu2 = nc.declare_dram_parameter("uu2", [2, KXP], BF, isOutput=False)
    wihm_t = nc.declare_dram_parameter("wihm_t", [P, KGX, H4P], WIHM_DT, isOutput=False)
    whh_t = nc.declare_dram_parameter("whh_t", [P, KT, H4P], F8, isOutput=False)
    wh1 = nc.declare_dram_parameter("wh1", [P, 4, F1], BF, isOutput=False)
    wh2 = nc.declare_dram_parameter("wh2", [P, F1 // P, F2], BF, isOutput=False)
    wh3 = nc.declare_dram_parameter("wh3", [P, F2 // P, F3], BF, isOutput=False)
    wh4 = nc.declare_dram_parameter("wh4", [P, F3 // P, OUT], BF, isOutput=False)
    bh1s = nc.declare_dram_parameter("bh1s", [P, F1 // P], F32, isOutput=False)
    bh2s = nc.declare_dram_parameter("bh2s", [P, F2 // P], F32, isOutput=False)
    bh3s = nc.declare_dram_parameter("bh3s", [P, F3 // P], F32, isOutput=False)
    bh4s = nc.declare_dram_parameter("bh4s", [OUT, 1], F32, isOutput=False)
    out = nc.declare_dram_parameter("out", [OUT, NLOC], F32, isOutput=True)

    p_dram = nc.dram_tensor("p_dram", [T, L], BF)
    q_dram = nc.dram_tensor("q_dram", [T, L], BF)

    with tile.TileContext(nc) as tc:
        with (
            tc.tile_pool(name="const", bufs=1) as constp,
            tc.tile_pool(name="state", bufs=1) as statep,
            tc.tile_pool(name="headw", bufs=1) as headwp,
        ):
            a_sb = constp.tile([L, L], F32)
            nc.sync.dma_start(out=a_sb, in_=a_hat[:, :])
            ident = constp.tile([128, 128], F32)
            make_identity(nc, ident)
            uu2_sb = constp.tile([2, KXP], BF)
            wihm_sb = constp.tile([P, KGX, H4P], WIHM_DT)
            whh_sb = constp.tile([P, KT, H4P], F8)

            wh1_sb = headwp.tile([P, 4, F1], BF)
            wh4_sb = headwp.tile([P, F3 // P, OUT], BF)
            bh1_sb = headwp.tile([P, F1 // P], F32)
            bh2_sb = headwp.tile([P, F2 // P], F32)
            bh3_sb = headwp.tile([P, F3 // P], F32)
            bh4_sb = headwp.tile([OUT, 1], F32)
            wh2_sb = headwp.tile([P, F1 // P, F2], BF)
            wh3_sb = headwp.tile([P, F2 // P, F3], BF)

            hT = statep.tile([P, KT, NLOC], BF)
            cT = statep.tile([P, KT, NLOC], BF)
            nc.vector.memset(hT, 0.0)
            nc.vector.memset(cT, 0.0)

            # ================= GCN =================
            with (
                tc.tile_pool(name="gcn", bufs=2) as gcnp,
                tc.tile_pool(name="gcn1", bufs=1) as gcn1p,
                tc.tile_pool(name="gcn_ps", bufs=2, space="PSUM") as gcnps,
            ):
                xT_sb = gcn1p.tile([L, T], F32)
                for i in range(2):
                    xt = gcnp.tile([96, L], F32, tag="xt")
                    nc.sync.dma_start(out=xt, in_=xb[i])
                    xT_ps = gcnps.tile([L, 96], F32, tag="tp")
                    nc.tensor.transpose(xT_ps, xt, ident[:96, :96])
                    nc.scalar.copy(xT_sb[:, i * 96 : (i + 1) * 96], xT_ps)
                mT_ps = gcnps.tile([L, T], F32, tag="mm")
                nc.tensor.matmul(mT_ps, lhsT=a_sb, rhs=xT_sb, start=True, stop=True)
                mp_sb = gcn1p.tile([L, T], F32)
                mm_sb = gcn1p.tile([L, T], F32)
                nc.scalar.activation(mp_sb, mT_ps, AF.Relu)
                nc.scalar.activation(mm_sb, mT_ps, AF.Relu, scale=-1.0)
                for src, dst in ((mp_sb, p_dram), (mm_sb, q_dram)):
                    rT_ps = gcnps.tile([L, T], F32, tag="mm")
                    nc.tensor.matmul(rT_ps, lhsT=a_sb, rhs=src, start=True, stop=True)
                    rT_sb = gcnp.tile([L, T], F32, tag="rt")
                    nc.scalar.copy(rT_sb, rT_ps)
                    for i in range(2):
                        r_ps = gcnps.tile([96, L], F32, tag="tp2")
                        nc.tensor.transpose(
                            r_ps, rT_sb[:, i * 96 : (i + 1) * 96], ident[:L, :L]
                        )
                        r_sb = gcnp.tile([96, L], BF, tag="rsb")
                        nc.scalar.copy(r_sb, r_ps)
                        nc.sync.dma_start(out=dst[i * 96 : (i + 1) * 96, :], in_=r_sb)

            nc.sync.dma_start(out=uu2_sb, in_=uu2[:, :])
            nc.sync.dma_start(out=wihm_sb, in_=wihm_t[:, :, :])
            nc.sync.dma_start(out=whh_sb, in_=whh_t[:, :, :])

            # ============ LSTM ============
            with (
                tc.tile_pool(name="pq", bufs=3) as pqp,
                tc.tile_pool(name="h2", bufs=2) as h2p,
                tc.tile_pool(name="ltmp", bufs=2) as ltp,
                tc.tile_pool(name="h2_ps", bufs=2, space="PSUM") as h2ps,
                tc.tile_pool(name="rec_ps", bufs=1, space="PSUM") as recps,
            ):
                h2_tiles = [None] * NCH
                pq_tiles = [None] * NCH

                def produce_pq(c):
                    pq = pqp.tile([2, CH, NLOC], BF, tag="pq", name="pq")
                    nc.sync.dma_start(
                        out=pq[0:1], in_=p_dram[c * CH : (c + 1) * CH, 0:NLOC][None]
                    )
                    nc.sync.dma_start(
                        out=pq[1:2], in_=q_dram[c * CH : (c + 1) * CH, 0:NLOC][None]
                    )
                    pq_tiles[c] = pq

                last_relu = [None]

                def produce_h2(c):
                    """h2c k-space = relu([p; q; 0; mixed]) + ones-row, chunk c."""
                    pq = pq_tiles[c]
                    h2 = h2p.tile([P, KGX, NPOS], BF, tag="h2")
                    for gt in range(KGX):
                        h2_ps = h2ps.tile([P, NPOS], F32, tag="h2ps")
                        nc.tensor.matmul(
                            h2_ps,
                            lhsT=uu2_sb[:, gt * P : (gt + 1) * P],
                            rhs=pq,
                            start=True,
                            stop=True,
                        )
                        last_relu[0] = nc.vector.tensor_scalar_max(h2[:, gt], h2_ps, 0.0)
                    nc.vector.memset(h2[0:1, 1, :], 1.0)  # bias ones-row (k-row 128)
                    h2_tiles[c] = h2

                produce_pq(0)
                produce_pq(1)
                produce_h2(0)

                # head weights ride the SWDGE queue, gated behind the LSTM
                # prologue so they can't starve identity/pq/p/q traffic
                from concourse.tile_rust import add_dep_helper

                for dst, src_ap in (
                    (wh1_sb, wh1[:, :, :]),
                    (wh4_sb, wh4[:, :, :]),
                    (bh1_sb, bh1s[:, :]),
                    (bh2_sb, bh2s[:, :]),
                    (bh3_sb, bh3s[:, :]),
                    (bh4_sb, bh4s[:, :]),
                    (wh2_sb, wh2[:, :, :]),
                    (wh3_sb, wh3[:, :, :]),
                ):
                    di = nc.gpsimd.dma_start(out=dst, in_=src_ap)
                    add_dep_helper(di.ins, last_relu[0].ins, sync=True, reason="delay head DMA")

                # device gate order in m: [F I G O]; psum pairs [F|I], [G|O].
                # Each bank's step is ONE accumulation group:
                #   [h2-part k-waves (no h dep -> fills prior step's tail),
                #    h-part k-waves (k01 then k23, matching sliced h writes)]
                for c in range(NCH):
                    h2c = h2_tiles[c]
                    for s in range(CH):
                        # one PSUM bank per gate, device m-order [G I F O]
                        ps = [
                            recps.tile([P, 4, 64], F32, tag=f"ps{i}", name=f"ps{i}")
                            for i in range(4)
                        ]
                        for g in range(4):
                            for k in range(KGX):
                                for mi in range(4):
                                    m = g * 4 + mi
                                    nc.tensor.matmul(
                                        ps[g][:, mi, 0:NLOC],
                                        lhsT=wihm_sb[:, k, m * P : (m + 1) * P],
                                        rhs=h2c[:, k, s * NLOC : (s + 1) * NLOC],
                                        start=(k == 0 and mi == 0),
                                        stop=False,
                                    )
                        for g in range(4):
                            for k in range(KT):
                                for mi in range(4):
                                    m = g * 4 + mi
                                    nc.tensor.matmul(
                                        ps[g][:, mi, 0:NLOC],
                                        lhsT=whh_sb[:, k, m * P : (m + 1) * P],
                                        rhs=hT[:, k],
                                        start=False,
                                        stop=(k == KT - 1 and mi == 3),
                                    )
                        # activations straight off PSUM (scale folds 1/512)
                        # bank0 = [G|I] (done first), bank1 = [F|O]
                        tg = ltp.tile([P, 4, NLOC], BF, tag="tg")
                        nc.scalar.activation(
                            tg, ps[0][:, 0:4, 0:NLOC], AF.Tanh, scale=1.0 / WSCALE
                        )
                        si = ltp.tile([P, 4, NLOC], BF, tag="si")
                        nc.scalar.activation(
                            si, ps[1][:, 0:4, 0:NLOC], AF.Sigmoid, scale=1.0 / WSCALE
                        )
                        t2 = ltp.tile([P, 4, NLOC], BF, tag="t2")
                        nc.vector.tensor_tensor(
                            t2.rearrange("p a n -> p (a n)"),
                            si.rearrange("p a n -> p (a n)"),
                            tg.rearrange("p a n -> p (a n)"),
                            op=ALU.mult,
                        )
                        sf = ltp.tile([P, 4, NLOC], BF, tag="sf")
                        nc.scalar.activation(
                            sf, ps[2][:, 0:4, 0:NLOC], AF.Sigmoid, scale=1.0 / WSCALE
                        )
                        t1 = ltp.tile([P, 4, NLOC], BF, tag="t1")
                        nc.vector.tensor_tensor(
                            t1.rearrange("p a n -> p (a n)"),
                            sf.rearrange("p a n -> p (a n)"),
                            cT.rearrange("p a n -> p (a n)"),
                            op=ALU.mult,
                        )
                        so = ltp.tile([P, 4, NLOC], BF, tag="so")
                        nc.scalar.activation(
                            so, ps[3][:, 0:4, 0:NLOC], AF.Sigmoid, scale=1.0 / WSCALE
                        )
                        nc.vector.tensor_tensor(
                            cT.rearrange("p a n -> p (a n)"),
                            t1.rearrange("p a n -> p (a n)"),
                            t2.rearrange("p a n -> p (a n)"),
                            op=ALU.add,
                        )
                        # sliced tanh(c)/h so next step's k01 waves start early
                        tc_ = ltp.tile([P, 4, NLOC], BF, tag="tc")
                        for j in range(2):
                            nc.scalar.activation(
                                tc_[:, 2 * j : 2 * j + 2], cT[:, 2 * j : 2 * j + 2], AF.Tanh
                            )
                            nc.vector.tensor_tensor(
                                hT[:, 2 * j : 2 * j + 2].rearrange("p a n -> p (a n)"),
                                so[:, 2 * j : 2 * j + 2].rearrange("p a n -> p (a n)"),
                                tc_[:, 2 * j : 2 * j + 2].rearrange("p a n -> p (a n)"),
                                op=ALU.mult,
                            )
                        # stage next chunk's inputs early in the chunk
                        if s == 0 and c + 1 < NCH:
                            if c + 2 < NCH:
                                produce_pq(c + 2)
                            produce_h2(c + 1)

            # ================= head =================
            with (
                tc.tile_pool(name="hd1", bufs=1) as hd1p,
                tc.tile_pool(name="hd_ps", bufs=4, space="PSUM") as hdps,
            ):
                z1 = hd1p.tile([P, F1 // P, NLOC], BF)
                for mp in range(F1 // P // 2):
                    ps = hdps.tile([P, 2, 64], F32, tag="zps")
                    for j in range(2):
                        m = 2 * mp + j
                        for k in range(4):
                            nc.tensor.matmul(
                                ps[:, j, 0:NLOC],
                                lhsT=wh1_sb[:, k, m * P : (m + 1) * P],
                                rhs=hT[:, k],
                                start=(k == 0 and j == 0),
                                stop=(k == 3 and j == 1),
                            )
                    for j in range(2):
                        m = 2 * mp + j
                        nc.scalar.activation(
                            z1[:, m], ps[:, j, 0:NLOC], AF.Relu, bias=bh1_sb[:, m : m + 1]
                        )
                z2 = hd1p.tile([P, F2 // P, NLOC], BF)
                for m in range(F2 // P):
                    ps = hdps.tile([P, NLOC], F32, tag="zps")
                    for k in range(F1 // P):
                        nc.tensor.matmul(
                            ps,
                            lhsT=wh2_sb[:, k, m * P : (m + 1) * P],
                            rhs=z1[:, k],
                            start=(k == 0),
                            stop=(k == F1 // P - 1),
                        )
                    nc.scalar.activation(
                        z2[:, m], ps, AF.Relu, bias=bh2_sb[:, m : m + 1]
                    )
                z3 = hd1p.tile([P, F3 // P, NLOC], BF)
                for m in range(F3 // P):
                    ps = hdps.tile([P, NLOC], F32, tag="zps")
                    for k in range(F2 // P):
                        nc.tensor.matmul(
                            ps,
                            lhsT=wh3_sb[:, k, m * P : (m + 1) * P],
                            rhs=z2[:, k],
                            start=(k == 0),
                            stop=(k == F2 // P - 1),
                        )
                    nc.scalar.activation(
                        z3[:, m], ps, AF.Relu, bias=bh3_sb[:, m : m + 1]
                    )
                ps4 = hdps.tile([OUT, NLOC], F32, tag="z4")
                for k in range(F3 // P):
                    nc.tensor.matmul(
                        ps4,
                        lhsT=wh4_sb[:, k],
                        rhs=z3[:, k],
                        start=(k == 0),
                        stop=(k == F3 // P - 1),
                    )
                y_sb = hd1p.tile([OUT, NLOC], F32)
                nc.scalar.activation(y_sb, ps4, AF.Sigmoid, bias=bh4_sb[:, 0:1])
                nc.sync.dma_start(out=out[:, :], in_=y_sb)

    nc.compile()
    return nc


_PROG = None
_LAST_RESULTS = None


def _get_program():
    global _PROG
    if _PROG is None:
        _PROG = _build_program(wihm_fp8=WIHM_FP8)
    return _PROG


GATE_PERM = (2, 0, 1, 3)  # device [g, i, f, o] from pytorch [i, f, g, o]


def _pad_gates(w, pad_in, pad_unit):
    H4_, K_ = w.shape
    hl = H4_ // 4
    out = np.zeros((4 * pad_unit, pad_in), w.dtype)
    for g in range(4):
        src = GATE_PERM[g]
        out[g * pad_unit : g * pad_unit + hl, :K_] = w[src * hl : (src + 1) * hl]
    return out


def _kstack(wT, p=P):
    K_, M_ = wT.shape
    return np.ascontiguousarray(wT.reshape(K_ // p, p, M_).transpose(1, 0, 2))


def _prep(
    x, A_hat, W1, W2, W_ih, W_hh, b_ih, b_hh, Wh1, bh1, Wh2, bh2, Wh3, bh3, Wh4, bh4
):
    f = np.float32
    u_plus = np.maximum(W1[0], 0) @ W2
    u_minus = np.maximum(-W1[0], 0) @ W2

    lin = (u_plus >= 0) & (u_minus >= 0)
    zer = (u_plus < 0) & (u_minus < 0)
    mix = ~(lin | zer)
    n_mix = int(mix.sum())
    n_drop = max(0, n_mix - (KXP - 3))  # 3 fixed rows: p, q, ones
    if n_drop > 0:
        m = np.einsum("ij,btj->bti", A_hat, x)
        p_mean = float(np.einsum("ij,btj->bti", A_hat, np.maximum(m, 0)).mean())
        q_mean = float(np.einsum("ij,btj->bti", A_hat, np.maximum(-m, 0)).mean())
        impact = np.where(
            mix,
            np.minimum(np.abs(u_plus) * p_mean, np.abs(u_minus) * q_mean),
            np.inf,
        )
        drop = np.argsort(impact)[:n_drop]
        lin = lin.copy()
        lin[drop] = True
        mix = mix.copy()
        mix[drop] = False
    mix_idx = np.nonzero(mix)[0]

    a_vec = W_ih @ (u_plus * lin)
    b_vec = W_ih @ (u_minus * lin)
    bias_vec = (b_ih + b_hh).astype(f)
    # k-row layout: 0=p, 1=q, 128=ones (partition 0 of k-tile 1, memset-able),
    # mixed features fill 2..127 and 129..255
    mix_cols = list(range(2, P)) + list(range(P + 1, KXP))
    wcat = np.zeros((4 * HL, KXP), f)
    wcat[:, 0] = a_vec
    wcat[:, 1] = b_vec
    wcat[:, P] = bias_vec
    for j, fidx in enumerate(mix_idx):
        wcat[:, mix_cols[j]] = W_ih[:, fidx]
    wcat_p = _pad_gates(wcat, KXP, HLP) * np.float32(WSCALE)
    WIHM_NP = FP8 if WIHM_FP8 else BF16
    wihm_t = _kstack(np.ascontiguousarray(wcat_p.T)).astype(WIHM_NP)

    uu2 = np.zeros((2, KXP), f)
    uu2[0, 0] = 1.0
    uu2[1, 1] = 1.0
    # col 128 stays 0 -> relu gives 0 -> memset writes the ones-row
    for j, fidx in enumerate(mix_idx):
        uu2[0, mix_cols[j]] = u_plus[fidx]
        uu2[1, mix_cols[j]] = u_minus[fidx]
    uu2 = uu2.astype(BF16)

    whh_p = _pad_gates(W_hh, HLP, HLP) * np.float32(WSCALE)
    whh_t = _kstack(np.ascontiguousarray(whh_p.T)).astype(FP8)

    def pad2(w, r, c):
        o = np.zeros((r, c), f)
        o[: w.shape[0], : w.shape[1]] = w
        return o

    wh1 = _kstack(pad2(Wh1, HLP, F1)).astype(BF16)
    wh2 = _kstack(pad2(Wh2, F1, F2)).astype(BF16)
    wh3 = _kstack(pad2(Wh3, F2, F3)).astype(BF16)
    wh4 = _kstack(pad2(Wh4, F3, OUT)).astype(BF16)
    bh1s = np.ascontiguousarray(pad2(bh1[None], 1, F1)[0].reshape(F1 // P, P).T)
    bh2s = np.ascontiguousarray(pad2(bh2[None], 1, F2)[0].reshape(F2 // P, P).T)
    bh3s = np.ascontiguousarray(pad2(bh3[None], 1, F3)[0].reshape(F3 // P, P).T)
    bh4s = np.ascontiguousarray(bh4.astype(f).reshape(OUT, 1))
    return uu2, wihm_t, whh_t, wh1, wh2, wh3, wh4, bh1s, bh2s, bh3s, bh4s


def prepare(
    x,
    A_hat,
    W1,
    W2,
    W_ih,
    W_hh,
    b_ih,
    b_hh,
    Wh1,
    bh1,
    Wh2,
    bh2,
    Wh3,
    bh3,
    Wh4,
    bh4,
):
    f = np.float32
    x = np.asarray(x, f)
    nc = _get_program()
    args = [
        np.asarray(a, f)
        for a in (W1, W2, W_ih, W_hh, b_ih, b_hh, Wh1, bh1, Wh2, bh2, Wh3, bh3, Wh4, bh4)
    ]
    a_hat = np.ascontiguousarray(np.asarray(A_hat, f))
    uu2, wihm_t, whh_t, wh1, wh2, wh3, wh4, bh1s, bh2s, bh3s, bh4s = _prep(
        x, a_hat, *args
    )

    a_roll = np.ascontiguousarray(np.roll(np.roll(a_hat, -NLOC, 0), -NLOC, 1))
    in_maps = []
    for c in range(NCORES):
        b = c // 2
        if c % 2 == 0:
            xc, ac = x[b], a_hat
        else:
            xc, ac = np.roll(x[b], -NLOC, axis=-1), a_roll
        in_maps.append(
            {
                "xb": np.ascontiguousarray(xc.reshape(2, 96, L)),
                "a_hat": ac,
                "uu2": uu2,
                "wihm_t": wihm_t,
                "whh_t": whh_t,
                "wh1": wh1,
                "wh2": wh2,
                "wh3": wh3,
                "wh4": wh4,
                "bh1s": bh1s,
                "bh2s": bh2s,
                "bh3s": bh3s,
                "bh4s": bh4s,
            }
        )
    return nc, in_maps


def assemble_output(res):
    y = np.zeros((B, OUT, L), np.float32)
    for c in range(NCORES):
        b = c // 2
        l0 = (c % 2) * NLOC
        y[b, :, l0 : l0 + NLOC] = res[c]["out"]
    return y


def kernel(**inputs):
    nc, in_maps = prepare(**inputs)
    global _LAST_RESULTS
    _LAST_RESULTS = run_bass_kernel_spmd(nc, in_maps, list(range(NCORES)))
    return assemble_output(_LAST_RESULTS.results)

